# revision 12
# baseline (speedup 1.0000x reference)
"""Trainium2 Bass kernel for nn_DPS_topk (topk_masking).

Math: the reference computes
    out = stop_gradient(hard - soft) + soft
which in the forward pass is numerically EXACTLY `hard` (the -soft and
+soft cancel bit-exactly in f32: positions where hard==0 give
(0 - s) + s == +0.0, and positions where hard==1 give (1 - s) + s which
rounds back to 1.0 for the tiny s produced here).  `hard` is the one-hot
expansion of the top-16 indices of (logits + gn) along D, with the k axis
ordered by ascending index.

Equivalently per row x (length D=1024):
    t   = 16th largest value of x
    m   = (x >= t)                         # membership mask, exactly 16 ones
    q   = inclusive_cumsum(m) * m          # rank 1..16 at selected, 0 else
    hard[j, d] = (q[d] == j + 1)

On-device algorithm per 128-row tile (rows on partitions, D on free axis):
    x   = gn_tile + logits_tile              (vector tensor_tensor add)
    v8  = max(x)                             (top-8 values, descending)
    x2  = match_replace(v8, x, -1e30)        (remove one instance of each)
    v16 = max(x2)                            (values ranked 9..16)
    m   = tensor_scalar(x, v16[:,7], is_ge)  (per-partition threshold)
    q   = tensor_tensor_scan(m, 0, add, add) (inclusive cumsum) * m
    out_j = tensor_scalar(q, j+1, is_equal)  for j in 0..15  -> (128, 16*1024)

Sharding: BS=32 split 4-per-core across 8 cores (data parallel); logits
replicated.  Per-core output (256, 16*1024) f32 = 16 MiB written as four
contiguous 4 MiB DMAs -> memory-bound at the ~358 GB/s HBM/core limit.

Raw Bass (no TileContext): this toolchain allows only ONE sync-wait
condition per instruction, which Tile's multi-wait tail drain violates.
Manual sems: every wait is a single wait_ge on a single semaphore.
"""

import numpy as np

K = 16
D = 1024
N = 64
BS = 32
NCORES = 8
BS_PER_CORE = BS // NCORES   # 4
ROWS = BS_PER_CORE * N       # 256 rows per core
P = 128                      # SBUF partitions
NTILES = ROWS // P           # 2
HALF = 8 * D                 # half a tile's output columns

_CACHE = {}


def _build_nc():
    import concourse.bass as bass
    from concourse import mybir

    f32 = mybir.dt.float32
    A = mybir.AluOpType

    nc = bass.Bass()
    lg_d = nc.declare_dram_parameter("logits", [N, D], f32, isOutput=False)
    gn_d = nc.declare_dram_parameter("gn", [ROWS, D], f32, isOutput=False)
    out_d = nc.declare_dram_parameter("out", [ROWS, K * D], f32, isOutput=True)

    with (
        nc.sbuf_tensor([P, NTILES * D], f32) as gn_all,
        nc.sbuf_tensor([P, D], f32) as lg,
        nc.sbuf_tensor([P, D], f32) as zeros,
        nc.sbuf_tensor([P, D], f32) as x,
        nc.sbuf_tensor([P, 8], f32) as v8,
        nc.sbuf_tensor([P, D], f32) as x2,
        nc.sbuf_tensor([P, 8], f32) as v16,
        nc.sbuf_tensor([P, D], f32) as m,
        nc.sbuf_tensor([P, D], f32) as qi,
        nc.sbuf_tensor([P, D], f32) as q,
        nc.sbuf_tensor([P, K * D], f32) as chunk0,
        nc.sbuf_tensor([P, K * D], f32) as chunk1,
        nc.semaphore("dma_sem") as dma_sem,
        nc.semaphore("cmp_sem") as cmp_sem,
        nc.Block() as block,
    ):
        chunks = [chunk0, chunk1]

        @block.sync
        def _(sync: "bass.BassEngine"):
            # inputs: one DMA for all of gn (partition p <- rows p, 128+p),
            # two for the replicated logits halves
            sync.dma_start(
                out=gn_all[:].rearrange("p (a d) -> p a d", a=NTILES),
                in_=gn_d[:, :].rearrange("(a p) d -> p a d", a=NTILES),
            ).then_inc(dma_sem, 16)
            sync.dma_start(out=lg[0:N, :], in_=lg_d[:, :]).then_inc(dma_sem, 16)
            sync.dma_start(out=lg[N:P, :], in_=lg_d[:, :]).then_inc(dma_sem, 16)

            # outputs: wait for each computed half-chunk, stream it out
            for i in range(NTILES):
                for h in range(2):
                    sync.wait_ge(cmp_sem, 2 * i + h + 1)
                    sync.dma_start(
                        out=out_d[P * i : P * (i + 1), HALF * h : HALF * (h + 1)],
                        in_=chunks[i][:, HALF * h : HALF * (h + 1)],
                    ).then_inc(dma_sem, 16)

            # all DMAs complete before kernel end
            sync.wait_ge(dma_sem, 16 * (3 + 2 * NTILES))

        @block.vector
        def _(vector: "bass.BassEngine"):
            # DVE writes are posted: a drain is required between a producing
            # op and a same-engine consumer of its output
            vector.memset(zeros[:], 0.0)
            vector.wait_ge(dma_sem, 48)  # all inputs resident
            for i in range(NTILES):
                vector.tensor_tensor(
                    x[:], gn_all[:, D * i : D * (i + 1)], lg[:], op=A.add
                )
                vector.drain()
                vector.max(v8[:], x[:])
                vector.drain()
                vector.match_replace(x2[:], v8[:], x[:], -1e30)
                vector.drain()
                vector.max(v16[:], x2[:])
                vector.drain()
                vector.tensor_scalar(m[:], x[:], v16[:, 7:8], None, op0=A.is_ge)
                vector.drain()
                vector.tensor_tensor_scan(
                    qi[:], m[:], zeros[:], 0.0, op0=A.add, op1=A.add
                )
                vector.drain()
                vector.tensor_tensor(q[:], qi[:], m[:], op=A.mult)
                vector.drain()
                for h in range(2):
                    for jj in range(8):
                        j = 8 * h + jj
                        vector.tensor_scalar(
                            chunks[i][:, D * j : D * (j + 1)],
                            q[:],
                            float(j + 1),
                            None,
                            op0=A.is_equal,
                        )
                    # flush the half-chunk, then signal the DMA engine
                    vector.drain().then_inc(cmp_sem, 1)

    return nc


def _get_nc():
    if "nc" not in _CACHE:
        _CACHE["nc"] = _build_nc()
    return _CACHE["nc"]


def kernel(logits: np.ndarray, gn: np.ndarray) -> np.ndarray:
    from concourse.bass_utils import run_bass_kernel_spmd

    logits = np.ascontiguousarray(np.asarray(logits, dtype=np.float32))
    gn = np.asarray(gn, dtype=np.float32)
    assert logits.shape == (N, D) and gn.shape == (BS, N, D)

    nc = _get_nc()
    in_maps = []
    for c in range(NCORES):
        shard = np.ascontiguousarray(
            gn[c * BS_PER_CORE : (c + 1) * BS_PER_CORE].reshape(ROWS, D)
        )
        in_maps.append({"logits": logits, "gn": shard})

    res = run_bass_kernel_spmd(nc, in_maps, list(range(NCORES))).results
    out = np.concatenate(
        [r["out"].reshape(BS_PER_CORE, N, K, D) for r in res], axis=0
    )
    return out.astype(np.float32, copy=False)


# revision 16
# speedup vs baseline: 1.0285x; 1.0285x over previous
"""Trainium2 Bass kernel for nn_DPS_topk (topk_masking).

Math: the reference computes
    out = stop_gradient(hard - soft) + soft
which in the forward pass is numerically EXACTLY `hard` (the -soft and
+soft cancel bit-exactly in f32: positions where hard==0 give
(0 - s) + s == +0.0, and positions where hard==1 give (1 - s) + s which
rounds back to 1.0 for the tiny s produced here).  `hard` is the one-hot
expansion of the top-16 indices of (logits + gn) along D, with the k axis
ordered by ascending index.

Equivalently per row x (length D=1024):
    t   = 16th largest value of x
    m   = (x >= t)                         # membership mask, exactly 16 ones
    q   = inclusive_cumsum(m) * m          # rank 1..16 at selected, 0 else
    hard[j, d] = (q[d] == j + 1)

On-device algorithm per 128-row tile (rows on partitions, D on free axis):
    x   = gn_tile + logits_tile              (vector tensor_tensor add)
    v8  = max(x)                             (top-8 values, descending)
    x2  = match_replace(v8, x, -1e30)        (remove one instance of each)
    v16 = max(x2)                            (values ranked 9..16)
    m   = tensor_scalar(x, v16[:,7], is_ge)  (per-partition threshold)
    q   = tensor_tensor_scan(m, 0, add, add) (inclusive cumsum) * m
    out_j = tensor_scalar(q, j+1, is_equal)  for j in 0..15  -> (128, 16*1024)

Sharding: BS=32 split 4-per-core across 8 cores (data parallel); logits
replicated.  Per-core output (256, 16*1024) f32 = 16 MiB streamed as
32 x 512 KiB DMAs, one per computed one-hot plane, so the write stream
starts as soon as the first plane exists -> memory-bound at the
~360-400 GB/s HBM/core limit.

Raw Bass (no TileContext): this toolchain allows only ONE sync-wait
condition per instruction, which Tile's multi-wait tail drain violates.
Manual sems: every wait is a single wait_ge on a single semaphore.
The DVE serializes same-engine dependent ops in hardware (per-op pipe
flush), so no explicit drains are needed; sem increments ride on the
producing instruction as in Tile-generated code.
"""

import numpy as np

K = 16
D = 1024
N = 64
BS = 32
NCORES = 8
BS_PER_CORE = BS // NCORES   # 4
ROWS = BS_PER_CORE * N       # 256 rows per core
P = 128                      # SBUF partitions
NTILES = ROWS // P           # 2

_CACHE = {}


def _build_nc():
    """Explicit DVE drains separate every dependent same-engine pair: raw
    Bass gets no automatic per-op pipeline drain (that is inserted by the
    Tile/bacc toolchain), and on hardware a dependent op issued before the
    producer's posted writes drain reads stale SBUF.  The drains overlap
    the producing op's streaming phase, so they cost only ~the pipe-empty
    tail (~0.3us) each."""
    import concourse.bass as bass
    from concourse import mybir

    f32 = mybir.dt.float32
    A = mybir.AluOpType

    nc = bass.Bass()
    lg_d = nc.declare_dram_parameter("logits", [N, D], f32, isOutput=False)
    gn_d = nc.declare_dram_parameter("gn", [ROWS, D], f32, isOutput=False)
    out_d = nc.declare_dram_parameter("out", [ROWS, K * D], f32, isOutput=True)

    with (
        nc.sbuf_tensor([P, D], f32) as gt0,
        nc.sbuf_tensor([P, D], f32) as gt1,
        nc.sbuf_tensor([P, D], f32) as lg,
        nc.sbuf_tensor([P, D], f32) as zeros,
        nc.sbuf_tensor([P, D], f32) as x,
        nc.sbuf_tensor([P, 8], f32) as v8,
        nc.sbuf_tensor([P, D], f32) as x2,
        nc.sbuf_tensor([P, 8], f32) as v16,
        nc.sbuf_tensor([P, D], f32) as m,
        nc.sbuf_tensor([P, D], f32) as qi,
        nc.sbuf_tensor([P, D], f32) as q,
        nc.sbuf_tensor([P, K * D], f32) as chunk0,
        nc.sbuf_tensor([P, K * D], f32) as chunk1,
        nc.semaphore("in0_sem") as in0_sem,
        nc.semaphore("in1_sem") as in1_sem,
        nc.semaphore("cmp_sem") as cmp_sem,
        nc.semaphore("dma_sem") as dma_sem,
        nc.Block() as block,
    ):
        gts = [gt0, gt1]
        chunks = [chunk0, chunk1]

        @block.sync
        def _(sync: "bass.BassEngine"):
            # tile-0 inputs first (gate chain-0 on in0_sem), tile-1 gn after
            sync.dma_start(out=lg[0:N, :], in_=lg_d[:, :]).then_inc(in0_sem, 16)
            sync.dma_start(out=lg[N:P, :], in_=lg_d[:, :]).then_inc(in0_sem, 16)
            sync.dma_start(out=gt0[:], in_=gn_d[0:P, :]).then_inc(in0_sem, 16)
            sync.dma_start(out=gt1[:], in_=gn_d[P : 2 * P, :]).then_inc(in1_sem, 16)

            # Stream each one-hot plane out as soon as it is SAFELY readable.
            # The eq op's sem-inc fires at instruction end, ~an op-length
            # before its posted writes finish draining, so gate plane j's DMA
            # on eq j+2 (two planes of slack); the final planes gate on the
            # per-tile drain whose inc fires only once the pipe is empty.
            SLACK = 2
            PER_TILE = K + 1  # 16 eq incs + 1 drain inc
            for i in range(NTILES):
                for j in range(K):
                    sync.wait_ge(
                        cmp_sem, PER_TILE * i + min(j + 1 + SLACK, PER_TILE)
                    )
                    sync.dma_start(
                        out=out_d[P * i : P * (i + 1), D * j : D * (j + 1)],
                        in_=chunks[i][:, D * j : D * (j + 1)],
                    ).then_inc(dma_sem, 16)

            # all output DMAs complete before kernel end
            sync.wait_ge(dma_sem, 16 * NTILES * K)

        @block.vector
        def _(vector: "bass.BassEngine"):
            def dr():
                vector.drain()

            vector.memset(zeros[:], 0.0)
            for i in range(NTILES):
                if i == 0:
                    vector.wait_ge(in0_sem, 48)
                else:
                    vector.wait_ge(in1_sem, 16)
                vector.tensor_tensor(x[:], gts[i][:], lg[:], op=A.add)
                dr()
                vector.max(v8[:], x[:])
                dr()
                vector.match_replace(x2[:], v8[:], x[:], -1e30)
                dr()
                vector.max(v16[:], x2[:])
                dr()
                vector.tensor_scalar(m[:], x[:], v16[:, 7:8], None, op0=A.is_ge)
                dr()
                vector.tensor_tensor_scan(
                    qi[:], m[:], zeros[:], 0.0, op0=A.add, op1=A.add
                )
                dr()
                vector.tensor_tensor(q[:], qi[:], m[:], op=A.mult)
                dr()
                for j in range(K):
                    vector.tensor_scalar(
                        chunks[i][:, D * j : D * (j + 1)],
                        q[:],
                        float(j + 1),
                        None,
                        op0=A.is_equal,
                    ).then_inc(cmp_sem, 1)
                # pipe-empty marker: gates the last SLACK planes' DMAs
                vector.drain().then_inc(cmp_sem, 1)

    return nc


def _get_nc():
    if "nc" not in _CACHE:
        _CACHE["nc"] = _build_nc()
    return _CACHE["nc"]


def kernel(logits: np.ndarray, gn: np.ndarray) -> np.ndarray:
    from concourse.bass_utils import run_bass_kernel_spmd

    logits = np.ascontiguousarray(np.asarray(logits, dtype=np.float32))
    gn = np.asarray(gn, dtype=np.float32)
    assert logits.shape == (N, D) and gn.shape == (BS, N, D)

    nc = _get_nc()
    in_maps = []
    for c in range(NCORES):
        shard = np.ascontiguousarray(
            gn[c * BS_PER_CORE : (c + 1) * BS_PER_CORE].reshape(ROWS, D)
        )
        in_maps.append({"logits": logits, "gn": shard})

    res = run_bass_kernel_spmd(nc, in_maps, list(range(NCORES))).results
    out = np.concatenate(
        [r["out"].reshape(BS_PER_CORE, N, K, D) for r in res], axis=0
    )
    return out.astype(np.float32, copy=False)
